# revision 1
# baseline (speedup 1.0000x reference)
"""Trainium2 Bass kernel for nn_CINLayer (3-layer CIN: chained bilinear einsums).

Strategy (data-parallel over batch, 8 cores x 512 rows):
  X1 = einsum('hjk,bjd,bkd->bhd', W0r, X0, X0); S1 = X1.sum(d)
  X2 = einsum(W1r, X0, X1);                     S2 = X2.sum(d)
  S3 = einsum over the Gram matrix G[b,j,k] = sum_d X0[b,j,d] X2[b,k,d]
       (final layer output only needs the d-sum, so X3 is never materialized)

Device layout: "c-major" Khatri-Rao product tiles P[(j,k), n] with n=(b,d),
built by DVE tensor-tensor multiplies against partition-broadcast rows of X0
(broadcast done by DMA from DRAM with stride-0 / row-replicating source APs),
consumed by the PE as accumulating matmuls. L0 packs 3 j's x 39 k per
117-partition chunk (no padding); L1 uses 39 chunks of (1 j x 128 k).
The last layer uses per-8-batch Gram matmuls (lhsT = DMA-transposed X2
tiles) against a block-diagonal host-built X0 rhs. Work proceeds in four
2048-column quarters with the Gram/S3 stage pipelined behind each L1
quarter.
"""

import sys

import numpy as np

try:
    import concourse.bass as bass  # noqa: F401
except ImportError:
    sys.path.insert(0, "/opt/trn_rl_repo")

import ml_dtypes

BF16 = ml_dtypes.bfloat16

B, F0, D, H = 4096, 39, 16, 128
N_CORES = 8
BC = B // N_CORES            # 512 batch rows per core
N = BC * D                   # 8192 columns, n = (b, d), d innermost
NQ = N // 4                  # 2048-column quarters (4 PSUM banks each)
C0_CHUNKS = 13               # j-triples: 3 j x 39 k = 117 rows per chunk
C0_ROWS = 117
C1_CHUNKS = 39               # 39 j's, k = 128 dense
NT8 = BC // 8                # 64 tiles of 8 batch rows (Gram)
GQ = 4                       # Gram/S3 quarters (128 b each), one per n-quarter

_CACHE = {}


def _build():
    import concourse.bass as bass
    import concourse.tile as tile
    from concourse import bacc, mybir

    bf16 = mybir.dt.bfloat16
    f32 = mybir.dt.float32
    AF = mybir.ActivationFunctionType
    AX = mybir.AxisListType

    nc = bacc.Bacc("TRN2", target_bir_lowering=False, debug=False,
                   num_devices=N_CORES)

    x0t_d = nc.dram_tensor("x0t", [F0, N], bf16, kind="ExternalInput")
    x0trip_d = nc.dram_tensor("x0trip", [C0_ROWS, N], bf16, kind="ExternalInput")
    w0_d = nc.dram_tensor("w0", [128, C0_CHUNKS, 128], bf16, kind="ExternalInput")
    w1_d = nc.dram_tensor("w1", [128, C1_CHUNKS, 128], bf16, kind="ExternalInput")
    w2_d = nc.dram_tensor("w2", [128, C1_CHUNKS, 128], bf16, kind="ExternalInput")
    x0bd_d = nc.dram_tensor("x0bd", [128, NT8 * 312], bf16, kind="ExternalInput")
    ones1_d = nc.dram_tensor("ones1", [1, 128], bf16, kind="ExternalInput")
    ones3_d = nc.dram_tensor("ones3", [3, C0_ROWS], bf16, kind="ExternalInput")
    b0_d = nc.dram_tensor("b0", [128, 1], f32, kind="ExternalInput")
    b1_d = nc.dram_tensor("b1", [128, 1], f32, kind="ExternalInput")
    s1_d = nc.dram_tensor("s1", [128, BC], f32, kind="ExternalOutput")
    s2_d = nc.dram_tensor("s2", [128, BC], f32, kind="ExternalOutput")
    s3_d = nc.dram_tensor("s3", [128, BC], f32, kind="ExternalOutput")

    from contextlib import ExitStack

    with tile.TileContext(nc) as tc, ExitStack() as ctx:
        const = ctx.enter_context(tc.tile_pool(name="const", bufs=1))
        bcpool = ctx.enter_context(tc.tile_pool(name="bc", bufs=8))
        ppool = ctx.enter_context(tc.tile_pool(name="pp", bufs=8))
        x2dtpool = ctx.enter_context(tc.tile_pool(name="x2dtp", bufs=2))
        x0bdpool = ctx.enter_context(tc.tile_pool(name="x0bdp", bufs=2))
        gpool = ctx.enter_context(tc.tile_pool(name="gp", bufs=2))
        rowpool = ctx.enter_context(tc.tile_pool(name="rowp", bufs=4))

        x0trip_t = const.tile([C0_ROWS, N], bf16)
        nc.sync.dma_start(out=x0trip_t[:], in_=x0trip_d.ap())
        w0_t = const.tile([128, C0_CHUNKS, 128], bf16)
        nc.sync.dma_start(out=w0_t[:], in_=w0_d.ap())
        w1_t = const.tile([128, C1_CHUNKS, 128], bf16)
        nc.sync.dma_start(out=w1_t[:], in_=w1_d.ap())
        w2_t = const.tile([128, C1_CHUNKS, 128], bf16)
        nc.sync.dma_start(out=w2_t[:], in_=w2_d.ap())
        ones1_t = const.tile([1, 128], bf16)
        nc.sync.dma_start(out=ones1_t[:], in_=ones1_d.ap())
        ones3_t = const.tile([3, C0_ROWS], bf16)
        nc.sync.dma_start(out=ones3_t[:], in_=ones3_d.ap())
        b0_t = const.tile([128, 1], f32)
        nc.sync.dma_start(out=b0_t[:], in_=b0_d.ap())
        b1_t = const.tile([128, 1], f32)
        nc.sync.dma_start(out=b1_t[:], in_=b1_d.ap())

        x1_t = const.tile([128, N], bf16)
        x2_t = const.tile([128, N], bf16)
        s1_sb = const.tile([128, BC], f32)
        s2_sb = const.tile([128, BC], f32)
        s3_sb = const.tile([128, BC], f32)

        layers = [
            (C0_CHUNKS, C0_ROWS, w0_t, x0trip_t, b0_t, x1_t, s1_sb),
            (C1_CHUNKS, 128, w1_t, x1_t, b1_t, x2_t, s2_sb),
        ]

        with tc.tile_pool(name="psum", bufs=1, space="PSUM") as psum:
            for li, (nchunks, nrows, w_t, in0_t, bias_t, xout_t, ssb_t) in \
                    enumerate(layers):
                for q in range(4):
                    c0 = q * NQ
                    acc = [psum.tile([128, 512], f32, tag="acc", bufs=6,
                                     name=f"acc_{li}_{q}_{t}")
                           for t in range(4)]
                    gsz = 1
                    groups = [list(range(s, min(s + gsz, nchunks)))
                              for s in range(0, nchunks, gsz)]
                    for grp in groups:
                        gb = len(grp)
                        in1x = bcpool.tile([nrows, gb, NQ], bf16, tag="bc",
                                           name=f"in1x_{li}_{q}_{grp[0]}")
                        if li == 0:
                            # rows 3c..3c+2 -> partitions 0-38/39-77/78-116
                            src = bass.AP(x0t_d, (3 * grp[0]) * N + c0,
                                          [[N, 3], [0, F0], [1, NQ]])
                        else:
                            src = bass.AP(x0t_d, grp[0] * N + c0,
                                          [[0, 128], [N, gb], [1, NQ]])
                        nc.sync.dma_start(out=in1x[:], in_=src)
                        for ci, c in enumerate(grp):
                            p = ppool.tile([nrows, NQ], bf16, tag="p")
                            nc.vector.tensor_mul(p[:], in0_t[:, c0:c0 + NQ],
                                                 in1x[:, ci, :])
                            for t in range(4):
                                nc.tensor.matmul(acc[t][:],
                                                 lhsT=w_t[0:nrows, c, :],
                                                 rhs=p[:, t * 512:(t + 1) * 512],
                                                 start=(c == 0),
                                                 stop=(c == nchunks - 1))
                    for t in range(4):
                        nc.scalar.activation(
                            xout_t[:, c0 + t * 512: c0 + (t + 1) * 512],
                            acc[t][:], AF.Identity, bias=bias_t[:], scale=1.0)
                        nc.vector.reduce_sum(
                            ssb_t[:, q * 128 + t * 32: q * 128 + (t + 1) * 32],
                            xout_t[:, c0 + t * 512: c0 + (t + 1) * 512]
                            .rearrange("p (b d) -> p b d", d=D),
                            axis=AX.X)

                    if li == 1:
                        # Gram + S3 for this quarter (128 batch rows)
                        x2dt = x2dtpool.tile([128, NT8 // GQ, 128], bf16,
                                             tag="x2dt", name=f"x2dt_{q}")
                        x0bdq = x0bdpool.tile([128, (NT8 // GQ) * 312], bf16,
                                              tag="x0bd", name=f"x0bdq_{q}")
                        nc.sync.dma_start(
                            out=x0bdq[:],
                            in_=x0bd_d.ap()[:, q * (NT8 // GQ) * 312:
                                            (q + 1) * (NT8 // GQ) * 312])
                        g = gpool.tile([128, F0, 128], bf16, tag="g",
                                       name=f"g_{q}")
                        for t16 in range(NT8 // GQ):
                            t = q * (NT8 // GQ) + t16
                            nc.sync.dma_start_transpose(
                                out=x2dt[:, t16, :],
                                in_=x2_t[:, t * 128:(t + 1) * 128])

                            psg = psum.tile([128, 312], f32, tag="gram", bufs=2,
                                            name=f"psg_{q}_{t16}")
                            nc.tensor.matmul(psg[:], lhsT=x2dt[:, t16, :],
                                             rhs=x0bdq[:, t16 * 312:(t16 + 1) * 312],
                                             start=True, stop=True)
                            nc.scalar.activation(
                                g[:, :, t16 * 8:(t16 + 1) * 8],
                                psg[:].rearrange("p (b j) -> p j b", b=8),
                                AF.Copy)
                        pss3 = psum.tile([128, 128], f32, tag="acc", bufs=6,
                                         name=f"pss3_{q}")
                        for j in range(F0):
                            nc.tensor.matmul(pss3[:], lhsT=w2_t[:, j, :],
                                             rhs=g[:, j, :],
                                             start=(j == 0), stop=(j == F0 - 1))
                        nc.scalar.activation(s3_sb[:, q * 128:(q + 1) * 128],
                                             pss3[:], AF.Copy)

        nc.sync.dma_start(out=s1_d.ap(), in_=s1_sb[:])
        nc.sync.dma_start(out=s2_d.ap(), in_=s2_sb[:])
        nc.sync.dma_start(out=s3_d.ap(), in_=s3_sb[:])

    nc.compile()
    return nc


def _prep_core(Xc, w0l, w1l, w2l, b0, b1):
    """Per-core input maps. Xc: [BC, F0, D] float32."""
    x0t = Xc.transpose(1, 0, 2).reshape(F0, N).astype(BF16)   # [j, (b,d)]
    x0trip = np.ascontiguousarray(np.tile(x0t, (3, 1)))       # [117, N]

    # block-diagonal Gram rhs: [128=(8b,16d), (t, 8b, 39j)]
    tmp = Xc.reshape(NT8, 8, F0, D).transpose(0, 1, 3, 2)     # [t, bb, d, j]
    arr = np.zeros((NT8, 8, D, 8, F0), dtype=BF16)
    idx = np.arange(8)
    arr[:, idx, :, idx, :] = tmp.transpose(1, 0, 2, 3).astype(BF16)
    x0bd = arr.reshape(NT8, 128, 312).transpose(1, 0, 2).reshape(128, NT8 * 312)
    x0bd = np.ascontiguousarray(x0bd)

    ones1 = np.ones((1, 128), dtype=BF16)
    ones3 = np.zeros((3, C0_ROWS), dtype=BF16)
    for r in range(3):
        ones3[r, r * F0:(r + 1) * F0] = 1
    return {
        "x0t": x0t, "x0trip": x0trip, "ones1": ones1, "ones3": ones3,
        "w0": w0l, "w1": w1l, "w2": w2l, "x0bd": x0bd,
        "b0": b0.reshape(128, 1).astype(np.float32),
        "b1": b1.reshape(128, 1).astype(np.float32),
    }


def _prep_weights(W0, W1, W2):
    # L0: chunk i = j-triple (3i, 3i+1, 3i+2), rows kk = (jj, k) in 3x39
    W0r = W0.reshape(H, F0, F0)
    w0l = np.zeros((128, C0_CHUNKS, 128), dtype=BF16)
    # [h, i, jj, k] -> [kk=(jj,k), i, h]
    w0l[:C0_ROWS] = (W0r.reshape(H, C0_CHUNKS, 3, F0)
                     .transpose(2, 3, 1, 0).reshape(C0_ROWS, C0_CHUNKS, H)
                     .astype(BF16))
    w1l = np.ascontiguousarray(
        W1.reshape(H, F0, 128).transpose(2, 1, 0).astype(BF16))
    w2l = np.ascontiguousarray(
        W2.reshape(H, F0, 128).transpose(2, 1, 0).astype(BF16))
    return w0l, w1l, w2l


def kernel(embedded_features, W0, b0, W1, b1, W2, b2):
    from concourse.bass_utils import run_bass_kernel_spmd

    X = np.asarray(embedded_features, dtype=np.float32)
    b0 = np.asarray(b0, dtype=np.float32)
    b1 = np.asarray(b1, dtype=np.float32)
    b2 = np.asarray(b2, dtype=np.float32)
    w0l, w1l, w2l = _prep_weights(np.asarray(W0, dtype=np.float32),
                                  np.asarray(W1, dtype=np.float32),
                                  np.asarray(W2, dtype=np.float32))

    if "nc" not in _CACHE:
        _CACHE["nc"] = _build()
    nc = _CACHE["nc"]

    in_maps = [
        _prep_core(X[c * BC:(c + 1) * BC], w0l, w1l, w2l, b0, b1)
        for c in range(N_CORES)
    ]
    res = run_bass_kernel_spmd(nc, in_maps, core_ids=list(range(N_CORES)))

    out = np.empty((B, 3 * H), dtype=np.float32)
    for c in range(N_CORES):
        r = res.results[c]
        sl = slice(c * BC, (c + 1) * BC)
        out[sl, 0:H] = r["s1"].T + D * b0[None, :]
        out[sl, H:2 * H] = r["s2"].T + D * b1[None, :]
        out[sl, 2 * H:3 * H] = r["s3"].T + D * b2[None, :]
    return out



# revision 7
# speedup vs baseline: 1.1527x; 1.1527x over previous
"""Trainium2 Bass kernel for nn_CINLayer (3-layer CIN: chained bilinear einsums).

Strategy (data-parallel over batch, 8 cores x 512 rows):
  X1 = einsum('hjk,bjd,bkd->bhd', W0r, X0, X0); S1 = X1.sum(d)
  X2 = einsum(W1r, X0, X1);                     S2 = X2.sum(d)
  S3 = einsum over the Gram matrix G[b,j,k] = sum_d X0[b,j,d] X2[b,k,d]
       (final layer output only needs the d-sum, so X3 is never materialized)

Device layout: "c-major" Khatri-Rao product tiles P[(j,k), n] with n=(b,d),
built by DVE tensor-tensor multiplies, consumed by the PE as accumulating
matmuls over 2048-column quarters.

L0 exploits symmetry of the x0(x)x0 outer product: the 780 unique (j,k<=j)
pairs are packed as 20 diagonal bands k=(j+delta)%39 (delta=0..19, off-diag
weights doubled host-side), 3 bands per 117-row chunk -> 7 chunks instead of
13. The j-side factor comes from the resident x0trip tile; the k-side band
gather is a plain strided DMA from a host-duplicated x0dup tensor (no
partition replication).

L1's per-j row broadcast (x0 row j replicated across the 128 k-partitions)
is split between DMA (stride-0-source replicating descriptors) and the
otherwise-idle GPSIMD/Pool engine's PartitionBroadcast op, relieving the DMA
engines which are the baseline bottleneck.

S2 is computed on the PE from the Gram stage's X2 transposes with a
block-diagonal ones matrix (instead of DVE reduces); S1 uses DVE reduces.
The last layer uses per-8-batch Gram matmuls (lhsT = DMA-transposed X2
tiles) against a block-diagonal host-built X0 rhs. Work proceeds in four
2048-column quarters with the Gram/S2/S3 stage pipelined behind each L1
quarter.
"""

import sys

import numpy as np

try:
    import concourse.bass as bass  # noqa: F401
except ImportError:
    sys.path.insert(0, "/opt/trn_rl_repo")

import ml_dtypes

BF16 = ml_dtypes.bfloat16

B, F0, D, H = 4096, 39, 16, 128
N_CORES = 8
BC = B // N_CORES            # 512 batch rows per core
N = BC * D                   # 8192 columns, n = (b, d), d innermost
NQ = N // 4                  # 2048-column quarters (4 PSUM banks each)
NBANDS = 20                  # L0 symmetric bands: k = (j + delta) % 39
C0_CHUNKS = 7                # 6 chunks x 3 bands + 1 chunk x 2 bands
C1_CHUNKS = 39               # 39 j's, k = 128 dense
NT8 = BC // 8                # 64 tiles of 8 batch rows (Gram)
GQ = 4                       # Gram/S3 quarters (128 b each), one per n-quarter

# j-indices whose L1 broadcast tile is built by the Pool engine
# (PartitionBroadcast) instead of DMA; tuned against the timeline model.
POOL_BCAST_JS = frozenset(j for j in range(C1_CHUNKS) if j % 3 == 1)

_CACHE = {}


def _build():
    import concourse.bass as bass
    import concourse.tile as tile
    from concourse import bacc, mybir

    bf16 = mybir.dt.bfloat16
    f32 = mybir.dt.float32
    AF = mybir.ActivationFunctionType
    AX = mybir.AxisListType

    nc = bacc.Bacc("TRN2", target_bir_lowering=False, debug=False,
                   num_devices=N_CORES)

    x0t_d = nc.dram_tensor("x0t", [F0, N], bf16, kind="ExternalInput")
    x0dup_d = nc.dram_tensor("x0dup", [F0 + NBANDS - 1, N], bf16,
                             kind="ExternalInput")
    x0trip_d = nc.dram_tensor("x0trip", [117, N], bf16, kind="ExternalInput")
    w0_d = nc.dram_tensor("w0", [128, C0_CHUNKS, 128], bf16, kind="ExternalInput")
    w1_d = nc.dram_tensor("w1", [128, C1_CHUNKS, 128], bf16, kind="ExternalInput")
    w2_d = nc.dram_tensor("w2", [128, C1_CHUNKS, 128], bf16, kind="ExternalInput")
    x0bd_d = nc.dram_tensor("x0bd", [128, NT8 * 312], bf16, kind="ExternalInput")
    onesbd_d = nc.dram_tensor("onesbd", [128, 8], bf16, kind="ExternalInput")
    b0_d = nc.dram_tensor("b0", [128, 1], f32, kind="ExternalInput")
    b1_d = nc.dram_tensor("b1", [128, 1], f32, kind="ExternalInput")
    s1_d = nc.dram_tensor("s1", [128, BC], f32, kind="ExternalOutput")
    s2_d = nc.dram_tensor("s2", [128, BC], f32, kind="ExternalOutput")
    s3_d = nc.dram_tensor("s3", [128, BC], f32, kind="ExternalOutput")

    from contextlib import ExitStack

    with tile.TileContext(nc) as tc, ExitStack() as ctx:
        const = ctx.enter_context(tc.tile_pool(name="const", bufs=1))
        in0pool = ctx.enter_context(tc.tile_pool(name="in0", bufs=3))
        bcpool = ctx.enter_context(tc.tile_pool(name="bc", bufs=5))
        p0pool = ctx.enter_context(tc.tile_pool(name="pp0", bufs=4))
        ppool = ctx.enter_context(tc.tile_pool(name="pp", bufs=5))
        stpool = ctx.enter_context(tc.tile_pool(name="st", bufs=4))
        x2dtpool = ctx.enter_context(tc.tile_pool(name="x2dtp", bufs=2))
        x0bdpool = ctx.enter_context(tc.tile_pool(name="x0bdp", bufs=2))
        gpool = ctx.enter_context(tc.tile_pool(name="gp", bufs=2))

        x0trip_t = const.tile([117, N], bf16)
        nc.sync.dma_start(out=x0trip_t[:], in_=x0trip_d.ap())
        w0_t = const.tile([128, C0_CHUNKS, 128], bf16)
        nc.sync.dma_start(out=w0_t[:], in_=w0_d.ap())
        w1_t = const.tile([128, C1_CHUNKS, 128], bf16)
        nc.sync.dma_start(out=w1_t[:], in_=w1_d.ap())
        w2_t = const.tile([128, C1_CHUNKS, 128], bf16)
        nc.sync.dma_start(out=w2_t[:], in_=w2_d.ap())
        onesbd_t = const.tile([128, 8], bf16)
        nc.sync.dma_start(out=onesbd_t[:], in_=onesbd_d.ap())
        b0_t = const.tile([128, 1], f32)
        nc.sync.dma_start(out=b0_t[:], in_=b0_d.ap())
        b1_t = const.tile([128, 1], f32)
        nc.sync.dma_start(out=b1_t[:], in_=b1_d.ap())

        x1_t = const.tile([128, N], bf16)
        x2_t = const.tile([128, N], bf16)
        s1_sb = const.tile([128, BC], f32)
        s2_sb = const.tile([128, BC], f32)
        s3_sb = const.tile([128, BC], f32)

        with tc.tile_pool(name="psum", bufs=1, space="PSUM") as psum:
            for li in range(2):
                for q in range(4):
                    c0 = q * NQ
                    acc = [psum.tile([128, 512], f32, tag="acc", bufs=5,
                                     name=f"acc_{li}_{q}_{t}")
                           for t in range(4)]
                    if li == 0:
                        # L0: 7 symmetric band-chunks (3 bands of k=(j+d)%39)
                        for c in range(C0_CHUNKS):
                            nb = 3 if c < C0_CHUNKS - 1 else NBANDS - 3 * (C0_CHUNKS - 1)
                            nrows = nb * F0
                            in0 = in0pool.tile([117, NQ], bf16, tag="in0",
                                               name=f"in0_{q}_{c}")
                            src = bass.AP(x0dup_d, (3 * c) * N + c0,
                                          [[N, nb], [N, F0], [1, NQ]])
                            nc.sync.dma_start(out=in0[:nrows], in_=src)
                            p = p0pool.tile([117, NQ], bf16, tag="p0")
                            nc.vector.tensor_mul(p[:nrows],
                                                 x0trip_t[:nrows, c0:c0 + NQ],
                                                 in0[:nrows])
                            for t in range(4):
                                nc.tensor.matmul(acc[t][:],
                                                 lhsT=w0_t[0:nrows, c, :],
                                                 rhs=p[:nrows, t * 512:(t + 1) * 512],
                                                 start=(c == 0),
                                                 stop=(c == C0_CHUNKS - 1))
                        for t in range(4):
                            nc.scalar.activation(
                                x1_t[:, c0 + t * 512: c0 + (t + 1) * 512],
                                acc[t][:], AF.Identity, bias=b0_t[:], scale=1.0)
                            nc.vector.reduce_sum(
                                s1_sb[:, q * 128 + t * 32: q * 128 + (t + 1) * 32],
                                x1_t[:, c0 + t * 512: c0 + (t + 1) * 512]
                                .rearrange("p (b d) -> p b d", d=D),
                                axis=AX.X)
                        continue

                    # L1: 39 j-chunks; broadcast via Pool or DMA
                    for j in range(C1_CHUNKS):
                        bc = bcpool.tile([128, NQ], bf16, tag="bc",
                                         name=f"bc_{q}_{j}")
                        if j in POOL_BCAST_JS:
                            # PartitionBroadcast needs its source at
                            # partition 0: stage row j there first.
                            st = stpool.tile([1, NQ], bf16, tag="st",
                                             name=f"st_{q}_{j}")
                            nc.sync.dma_start(
                                out=st[:],
                                in_=bass.AP(x0t_d, j * N + c0,
                                            [[N, 1], [1, NQ]]))
                            nc.gpsimd.partition_broadcast(bc[:], st[0:1, :])
                        else:
                            src = bass.AP(x0t_d, j * N + c0,
                                          [[0, 128], [1, NQ]])
                            nc.sync.dma_start(out=bc[:], in_=src)
                        p = ppool.tile([128, NQ], bf16, tag="p")
                        nc.vector.tensor_mul(p[:], x1_t[:, c0:c0 + NQ], bc[:])
                        for t in range(4):
                            nc.tensor.matmul(acc[t][:],
                                             lhsT=w1_t[:, j, :],
                                             rhs=p[:, t * 512:(t + 1) * 512],
                                             start=(j == 0),
                                             stop=(j == C1_CHUNKS - 1))
                    for t in range(4):
                        nc.scalar.activation(
                            x2_t[:, c0 + t * 512: c0 + (t + 1) * 512],
                            acc[t][:], AF.Identity, bias=b1_t[:], scale=1.0)

                    # Gram + S2 + S3 for this quarter (128 batch rows)
                    x2dt = x2dtpool.tile([128, NT8 // GQ, 128], bf16,
                                         tag="x2dt", name=f"x2dt_{q}")
                    x0bdq = x0bdpool.tile([128, (NT8 // GQ) * 312], bf16,
                                          tag="x0bd", name=f"x0bdq_{q}")
                    nc.sync.dma_start(
                        out=x0bdq[:],
                        in_=x0bd_d.ap()[:, q * (NT8 // GQ) * 312:
                                        (q + 1) * (NT8 // GQ) * 312])
                    g = gpool.tile([128, F0, 128], bf16, tag="g",
                                   name=f"g_{q}")
                    s2ps = psum.tile([128, 128], f32, tag="s2", bufs=1,
                                     name=f"s2ps_{q}")
                    for t16 in range(NT8 // GQ):
                        t = q * (NT8 // GQ) + t16
                        nc.sync.dma_start_transpose(
                            out=x2dt[:, t16, :],
                            in_=x2_t[:, t * 128:(t + 1) * 128])

                        psg = psum.tile([128, 312], f32, tag="gram", bufs=2,
                                        name=f"psg_{q}_{t16}")
                        nc.tensor.matmul(psg[:], lhsT=x2dt[:, t16, :],
                                         rhs=x0bdq[:, t16 * 312:(t16 + 1) * 312],
                                         start=True, stop=True)
                        nc.tensor.matmul(s2ps[:, t16 * 8:(t16 + 1) * 8],
                                         lhsT=x2dt[:, t16, :],
                                         rhs=onesbd_t[:],
                                         start=True, stop=True)
                        nc.scalar.activation(
                            g[:, :, t16 * 8:(t16 + 1) * 8],
                            psg[:].rearrange("p (b j) -> p j b", b=8),
                            AF.Copy)
                    nc.scalar.activation(s2_sb[:, q * 128:(q + 1) * 128],
                                         s2ps[:], AF.Copy)
                    pss3 = psum.tile([128, 128], f32, tag="s2", bufs=1,
                                     name=f"pss3_{q}")
                    for j in range(F0):
                        nc.tensor.matmul(pss3[:], lhsT=w2_t[:, j, :],
                                         rhs=g[:, j, :],
                                         start=(j == 0), stop=(j == F0 - 1))
                    nc.scalar.activation(s3_sb[:, q * 128:(q + 1) * 128],
                                         pss3[:], AF.Copy)

        nc.sync.dma_start(out=s1_d.ap(), in_=s1_sb[:])
        nc.sync.dma_start(out=s2_d.ap(), in_=s2_sb[:])
        nc.sync.dma_start(out=s3_d.ap(), in_=s3_sb[:])

    nc.compile()
    return nc


def _prep_core(Xc, w0l, w1l, w2l, b0, b1):
    """Per-core input maps. Xc: [BC, F0, D] float32."""
    x0t = Xc.transpose(1, 0, 2).reshape(F0, N).astype(BF16)   # [j, (b,d)]
    x0dup = np.ascontiguousarray(
        np.concatenate([x0t, x0t[:NBANDS - 1]], axis=0))      # [58, N]
    x0trip = np.ascontiguousarray(np.tile(x0t, (3, 1)))       # [117, N]

    # block-diagonal Gram rhs: [128=(8b,16d), (t, 8b, 39j)]
    tmp = Xc.reshape(NT8, 8, F0, D).transpose(0, 1, 3, 2)     # [t, bb, d, j]
    arr = np.zeros((NT8, 8, D, 8, F0), dtype=BF16)
    idx = np.arange(8)
    arr[:, idx, :, idx, :] = tmp.transpose(1, 0, 2, 3).astype(BF16)
    x0bd = arr.reshape(NT8, 128, 312).transpose(1, 0, 2).reshape(128, NT8 * 312)
    x0bd = np.ascontiguousarray(x0bd)

    # block-diagonal ones for S2: [(8b',16d), 8b] -> 1 iff b'==b
    onesbd = np.zeros((128, 8), dtype=BF16)
    for b in range(8):
        onesbd[b * D:(b + 1) * D, b] = 1

    return {
        "x0t": x0t, "x0dup": x0dup, "x0trip": x0trip,
        "w0": w0l, "w1": w1l, "w2": w2l, "x0bd": x0bd, "onesbd": onesbd,
        "b0": b0.reshape(128, 1).astype(np.float32),
        "b1": b1.reshape(128, 1).astype(np.float32),
    }


def _prep_weights(W0, W1, W2):
    # L0 symmetric bands: chunk c, band-in-chunk i -> delta = 3c+i,
    # row kk = i*39 + j pairs x0[j] (from x0trip) with x0[(j+delta)%39]
    # (from x0dup). Off-diagonal weights doubled (each unordered pair once).
    W0r = W0.reshape(H, F0, F0)
    W0sym = W0r + W0r.transpose(0, 2, 1)
    jj = np.arange(F0)
    w0l = np.zeros((128, C0_CHUNKS, 128), dtype=BF16)
    for delta in range(NBANDS):
        c, i = divmod(delta, 3)
        kk = i * F0 + jj
        kcol = (jj + delta) % F0
        vals = W0r[:, jj, jj] if delta == 0 else W0sym[:, jj, kcol]
        w0l[kk, c, :] = vals.T.astype(BF16)
    w1l = np.ascontiguousarray(
        W1.reshape(H, F0, 128).transpose(2, 1, 0).astype(BF16))
    w2l = np.ascontiguousarray(
        W2.reshape(H, F0, 128).transpose(2, 1, 0).astype(BF16))
    return w0l, w1l, w2l


def kernel(embedded_features, W0, b0, W1, b1, W2, b2):
    from concourse.bass_utils import run_bass_kernel_spmd

    X = np.asarray(embedded_features, dtype=np.float32)
    b0 = np.asarray(b0, dtype=np.float32)
    b1 = np.asarray(b1, dtype=np.float32)
    b2 = np.asarray(b2, dtype=np.float32)
    w0l, w1l, w2l = _prep_weights(np.asarray(W0, dtype=np.float32),
                                  np.asarray(W1, dtype=np.float32),
                                  np.asarray(W2, dtype=np.float32))

    if "nc" not in _CACHE:
        _CACHE["nc"] = _build()
    nc = _CACHE["nc"]

    in_maps = [
        _prep_core(X[c * BC:(c + 1) * BC], w0l, w1l, w2l, b0, b1)
        for c in range(N_CORES)
    ]
    res = run_bass_kernel_spmd(nc, in_maps, core_ids=list(range(N_CORES)))

    out = np.empty((B, 3 * H), dtype=np.float32)
    for c in range(N_CORES):
        r = res.results[c]
        sl = slice(c * BC, (c + 1) * BC)
        # s1/s2 already include the bias (added per-d on device: D*b total);
        # s3 is computed bias-free via the Gram trick, add D*b2 here.
        out[sl, 0:H] = r["s1"].T
        out[sl, H:2 * H] = r["s2"].T
        out[sl, 2 * H:3 * H] = r["s3"].T + D * b2[None, :]
    return out


# revision 10
# speedup vs baseline: 1.3506x; 1.1717x over previous
"""Trainium2 Bass kernel for nn_CINLayer (3-layer CIN: chained bilinear einsums).

Strategy (data-parallel over batch, 8 cores x 512 rows):
  X1 = einsum('hjk,bjd,bkd->bhd', W0r, X0, X0); S1 = X1.sum(d)
  X2 = einsum(W1r, X0, X1);                     S2 = X2.sum(d)
  S3 = einsum over the Gram matrix G[b,j,k] = sum_d X0[b,j,d] X2[b,k,d]
       (final layer output only needs the d-sum, so X3 is never materialized)

Device layout: "c-major" Khatri-Rao product tiles P[(j,k), n] with n=(b,d),
built by DVE tensor-tensor multiplies, consumed by the PE as accumulating
matmuls over 2048-column quarters. L0(q) and L1(q) are interleaved per
quarter (L1(q) only needs X1 columns of quarter q) so all engines stay fed
from the start, and the Gram/S2/S3 stage for quarter q-1 is pipelined behind
L1(q)'s chunk loop.

L0 exploits symmetry of the x0(x)x0 outer product: the 780 unique (j,k<=j)
pairs are packed as 20 diagonal bands k=(j+delta)%39 (delta=0..19, off-diag
weights doubled host-side), 3 bands per 117-row chunk -> 7 chunks instead of
13. The j-side factor comes from resident x0trip tiles; the k-side band
gather is a plain strided DMA from a host-duplicated x0dup tensor.

L1's per-j row broadcast (x0 row j replicated across the 128 k-partitions)
is split between DMA (stride-0-source replicating descriptors) and the
otherwise-idle GPSIMD/Pool engine's PartitionBroadcast op (whose source must
sit on partition 0, hence tiny staging DMAs), relieving the DMA engines
which are the baseline bottleneck.

The Gram stage transposes a full quarter of X2 in a single tiled
dma_start_transpose ([128, 2048] -> 16 x [128,128] blocks), multiplies
against a block-diagonal X0 rhs assembled on-chip from a compact host tensor
(8 strided DMAs into a memset-once tile; the zero pattern is invariant
across quarters), and computes S2 on the PE from the same transposes with a
block-diagonal ones matrix. S1 uses DVE reduces.
"""

import sys

import numpy as np

try:
    import concourse.bass as bass  # noqa: F401
except ImportError:
    sys.path.insert(0, "/opt/trn_rl_repo")

import ml_dtypes

BF16 = ml_dtypes.bfloat16

B, F0, D, H = 4096, 39, 16, 128
N_CORES = 8
BC = B // N_CORES            # 512 batch rows per core
N = BC * D                   # 8192 columns, n = (b, d), d innermost
NQ = N // 4                  # 2048-column quarters (4 PSUM banks each)
NBANDS = 20                  # L0 symmetric bands: k = (j + delta) % 39
C0_CHUNKS = 7                # 6 chunks x 3 bands + 1 chunk x 2 bands
C1_CHUNKS = 39               # 39 j's, k = 128 dense
NT8 = BC // 8                # 64 tiles of 8 batch rows (Gram)
TQ = NT8 // 4                # 16 Gram tiles per quarter

# j-indices whose L1 broadcast tile is built by the Pool engine
# (PartitionBroadcast) instead of DMA; tuned against the timeline model.
POOL_BCAST_JS = frozenset({0, 18, 36} | {j for j in range(C1_CHUNKS)
                                         if j % 3 == 1})

_CACHE = {}


def _build():
    import concourse.bass as bass
    import concourse.tile as tile
    from concourse import bacc, mybir

    bf16 = mybir.dt.bfloat16
    f32 = mybir.dt.float32
    AF = mybir.ActivationFunctionType
    AX = mybir.AxisListType

    nc = bacc.Bacc("TRN2", target_bir_lowering=False, debug=False,
                   num_devices=N_CORES)

    x0t_d = nc.dram_tensor("x0t", [F0, N], bf16, kind="ExternalInput")
    x0dup_d = nc.dram_tensor("x0dup", [F0 + NBANDS - 1, N], bf16,
                             kind="ExternalInput")
    x0trip_d = nc.dram_tensor("x0trip", [117, N], bf16, kind="ExternalInput")
    w0_d = nc.dram_tensor("w0", [128, C0_CHUNKS, 128], bf16, kind="ExternalInput")
    w1_d = nc.dram_tensor("w1", [128, C1_CHUNKS, 128], bf16, kind="ExternalInput")
    w2_d = nc.dram_tensor("w2", [128, C1_CHUNKS, 128], bf16, kind="ExternalInput")
    x0c_d = nc.dram_tensor("x0c", [128, NT8, F0], bf16, kind="ExternalInput")
    onesbd_d = nc.dram_tensor("onesbd", [128, 8], bf16, kind="ExternalInput")
    b0_d = nc.dram_tensor("b0", [128, 1], f32, kind="ExternalInput")
    b1_d = nc.dram_tensor("b1", [128, 1], f32, kind="ExternalInput")
    s1_d = nc.dram_tensor("s1", [128, BC], f32, kind="ExternalOutput")
    s2_d = nc.dram_tensor("s2", [128, BC], f32, kind="ExternalOutput")
    s3_d = nc.dram_tensor("s3", [128, BC], f32, kind="ExternalOutput")

    from contextlib import ExitStack

    with tile.TileContext(nc) as tc, ExitStack() as ctx:
        const = ctx.enter_context(tc.tile_pool(name="const", bufs=1))
        in0pool = ctx.enter_context(tc.tile_pool(name="in0", bufs=4))
        x0trippool = ctx.enter_context(tc.tile_pool(name="x0trip", bufs=2))
        x1pool = ctx.enter_context(tc.tile_pool(name="x1p", bufs=2))
        x2pool = ctx.enter_context(tc.tile_pool(name="x2p", bufs=2))
        bcpool = ctx.enter_context(tc.tile_pool(name="bc", bufs=5))
        bc2pool = ctx.enter_context(tc.tile_pool(name="bc2", bufs=4))
        p0pool = ctx.enter_context(tc.tile_pool(name="pp0", bufs=4))
        ppool = ctx.enter_context(tc.tile_pool(name="pp", bufs=6))
        stpool = ctx.enter_context(tc.tile_pool(name="st", bufs=3))
        x2dtpool = ctx.enter_context(tc.tile_pool(name="x2dtp", bufs=1))
        gpool = ctx.enter_context(tc.tile_pool(name="gp", bufs=1))

        w0_t = const.tile([128, C0_CHUNKS, 128], bf16)
        w1_t = const.tile([128, C1_CHUNKS, 128], bf16)
        w2_t = const.tile([128, C1_CHUNKS, 128], bf16)
        onesbd_t = const.tile([128, 8], bf16)
        b0_t = const.tile([128, 1], f32)
        b1_t = const.tile([128, 1], f32)
        x0bd_t = [const.tile([128, TQ, 312], bf16, name=f"x0bd_{i}")
                  for i in range(2)]

        s1_sb = const.tile([128, BC], f32)
        s2_sb = const.tile([128, BC], f32)
        s3_sb = const.tile([128, BC], f32)

        # startup: only what L0(0) needs first, then the rest
        nc.sync.dma_start(out=w0_t[:], in_=w0_d.ap())
        nc.sync.dma_start(out=b0_t[:], in_=b0_d.ap())
        # block-diagonal Gram rhs holders: zero once; the nonzero diagonal
        # runs are rewritten per quarter, zeros stay zero.
        nc.gpsimd.memset(x0bd_t[0][:], 0)
        nc.gpsimd.memset(x0bd_t[1][:], 0)

        def gram_stage(qq, x2dt, psum):
            """Gram + S2 + S3 for quarter qq (transpose already issued)."""
            g = gpool.tile([128, F0, 128], bf16, tag="g", name=f"g_{qq}")
            s2ps = psum.tile([128, 128], f32, tag="s2", bufs=1,
                             name=f"s2ps_{qq}")
            xb = x0bd_t[qq % 2]
            for t16 in range(TQ):
                psg = psum.tile([128, 312], f32, tag="gram", bufs=2,
                                name=f"psg_{qq}_{t16}")
                nc.tensor.matmul(psg[:], lhsT=x2dt[:, t16, :],
                                 rhs=xb[:, t16, :],
                                 start=True, stop=True)
                nc.tensor.matmul(s2ps[:, t16 * 8:(t16 + 1) * 8],
                                 lhsT=x2dt[:, t16, :],
                                 rhs=onesbd_t[:],
                                 start=True, stop=True)
                nc.scalar.activation(
                    g[:, :, t16 * 8:(t16 + 1) * 8],
                    psg[:].rearrange("p (b j) -> p j b", b=8),
                    AF.Copy)
            nc.scalar.activation(s2_sb[:, qq * 128:(qq + 1) * 128],
                                 s2ps[:], AF.Copy)
            pss3 = psum.tile([128, 128], f32, tag="s2", bufs=1,
                             name=f"pss3_{qq}")
            for j in range(F0):
                nc.tensor.matmul(pss3[:], lhsT=w2_t[:, j, :],
                                 rhs=g[:, j, :],
                                 start=(j == 0), stop=(j == F0 - 1))
            nc.scalar.activation(s3_sb[:, qq * 128:(qq + 1) * 128],
                                 pss3[:], AF.Copy)

        with tc.tile_pool(name="psum", bufs=1, space="PSUM") as psum:
            for q in range(4):
                c0 = q * NQ
                x0trip_q = x0trippool.tile([117, NQ], bf16, tag="x0trip",
                                           name=f"x0trip_{q}")
                nc.sync.dma_start(out=x0trip_q[:],
                                  in_=x0trip_d.ap()[:, c0:c0 + NQ])

                # ---- L0 quarter q: 7 symmetric band-chunks ----
                x1q = x1pool.tile([128, NQ], bf16, tag="x1",
                                  name=f"x1_{q}")
                acc = [psum.tile([128, 512], f32, tag="acc", bufs=5,
                                 name=f"acc0_{q}_{t}")
                       for t in range(4)]
                for c in range(C0_CHUNKS):
                    nb = 3 if c < C0_CHUNKS - 1 else NBANDS - 3 * (C0_CHUNKS - 1)
                    nrows = nb * F0
                    in0 = in0pool.tile([117, NQ], bf16, tag="in0",
                                       name=f"in0_{q}_{c}")
                    src = bass.AP(x0dup_d, (3 * c) * N + c0,
                                  [[N, nb], [N, F0], [1, NQ]])
                    nc.sync.dma_start(out=in0[:nrows], in_=src)
                    p = p0pool.tile([117, NQ], bf16, tag="p0")
                    nc.vector.tensor_mul(p[:nrows],
                                         x0trip_q[:nrows, :],
                                         in0[:nrows])
                    for t in range(4):
                        nc.tensor.matmul(acc[t][:],
                                         lhsT=w0_t[0:nrows, c, :],
                                         rhs=p[:nrows, t * 512:(t + 1) * 512],
                                         start=(c == 0),
                                         stop=(c == C0_CHUNKS - 1))
                if q == 0:
                    nc.sync.dma_start(out=w1_t[:], in_=w1_d.ap())
                    nc.sync.dma_start(out=b1_t[:], in_=b1_d.ap())
                    nc.sync.dma_start(out=onesbd_t[:], in_=onesbd_d.ap())
                for t in range(4):
                    nc.scalar.activation(
                        x1q[:, t * 512:(t + 1) * 512],
                        acc[t][:], AF.Identity, bias=b0_t[:], scale=1.0)
                    nc.vector.reduce_sum(
                        s1_sb[:, q * 128 + t * 32: q * 128 + (t + 1) * 32],
                        x1q[:, t * 512:(t + 1) * 512]
                        .rearrange("p (b d) -> p b d", d=D),
                        axis=AX.X)

                # block-diagonal Gram rhs for quarter q: 8 strided DMAs
                # (one per 8-batch lane group) from the compact x0c tensor
                xb = x0bd_t[q % 2]
                for bp in range(8):
                    nc.sync.dma_start(
                        out=xb[bp * 16:(bp + 1) * 16, :,
                               bp * F0:(bp + 1) * F0],
                        in_=x0c_d.ap()[bp * 16:(bp + 1) * 16,
                                       q * TQ:(q + 1) * TQ, :])

                # ---- L1 quarter q: 39 j-chunks ----
                acc = [psum.tile([128, 512], f32, tag="acc", bufs=5,
                                 name=f"acc1_{q}_{t}")
                       for t in range(4)]
                x2q = x2pool.tile([128, NQ], bf16, tag="x2",
                                  name=f"x2_{q}")
                x2dt = None
                pair_tiles = {}
                for j in range(C1_CHUNKS):
                    if j in POOL_BCAST_JS:
                        # PartitionBroadcast needs its source at partition 0:
                        # stage row j there first.
                        bc = bcpool.tile([128, NQ], bf16, tag="bc",
                                         name=f"bc_{q}_{j}")
                        st = stpool.tile([1, NQ], bf16, tag="st",
                                         name=f"st_{q}_{j}")
                        nc.sync.dma_start(
                            out=st[:],
                            in_=bass.AP(x0t_d, j * N + c0, [[N, 1], [1, NQ]]))
                        nc.gpsimd.partition_broadcast(bc[:], st[0:1, :])
                        bcap = bc[:]
                    elif j in pair_tiles:
                        bcap = pair_tiles.pop(j)
                    elif (j + 1 < C1_CHUNKS and j + 1 not in POOL_BCAST_JS):
                        # two adjacent DMA-broadcast rows in one descriptor set
                        bc2 = bc2pool.tile([128, 2, NQ], bf16, tag="bc2",
                                           name=f"bc2_{q}_{j}")
                        src = bass.AP(x0t_d, j * N + c0,
                                      [[0, 128], [N, 2], [1, NQ]])
                        nc.sync.dma_start(out=bc2[:], in_=src)
                        pair_tiles[j + 1] = bc2[:, 1, :]
                        bcap = bc2[:, 0, :]
                    else:
                        bc = bcpool.tile([128, NQ], bf16, tag="bc",
                                         name=f"bc_{q}_{j}")
                        src = bass.AP(x0t_d, j * N + c0, [[0, 128], [1, NQ]])
                        nc.sync.dma_start(out=bc[:], in_=src)
                        bcap = bc[:]
                    p = ppool.tile([128, NQ], bf16, tag="p")
                    nc.vector.tensor_mul(p[:], x1q[:], bcap)
                    for t in range(4):
                        nc.tensor.matmul(acc[t][:],
                                         lhsT=w1_t[:, j, :],
                                         rhs=p[:, t * 512:(t + 1) * 512],
                                         start=(j == 0),
                                         stop=(j == C1_CHUNKS - 1))
                    if j == 1 and q > 0:
                        # transpose the whole previous quarter of X2 in one
                        # tiled DMA-transpose (16 x [128,128] blocks)
                        x2dt = x2dtpool.tile([128, TQ, 128], bf16,
                                             tag="x2dt", name=f"x2dt_{q - 1}")
                        nc.sync.dma_start_transpose(
                            out=x2dt[:], in_=x2prev[:])
                for t in range(4):
                    nc.scalar.activation(
                        x2q[:, t * 512:(t + 1) * 512],
                        acc[t][:], AF.Identity, bias=b1_t[:], scale=1.0)
                x2prev = x2q
                if q == 0:
                    nc.sync.dma_start(out=w2_t[:], in_=w2_d.ap())
                if q > 0:
                    gram_stage(q - 1, x2dt, psum)

            # tail: Gram for the last quarter
            x2dt = x2dtpool.tile([128, TQ, 128], bf16, tag="x2dt",
                                 name="x2dt_3")
            nc.sync.dma_start_transpose(out=x2dt[:], in_=x2prev[:])
            gram_stage(3, x2dt, psum)

        nc.sync.dma_start(out=s1_d.ap(), in_=s1_sb[:])
        nc.sync.dma_start(out=s2_d.ap(), in_=s2_sb[:])
        nc.sync.dma_start(out=s3_d.ap(), in_=s3_sb[:])

    nc.compile()
    return nc


def _prep_core(Xc, w0l, w1l, w2l, b0, b1):
    """Per-core input maps. Xc: [BC, F0, D] float32."""
    x0t = Xc.transpose(1, 0, 2).reshape(F0, N).astype(BF16)   # [j, (b,d)]
    x0dup = np.ascontiguousarray(
        np.concatenate([x0t, x0t[:NBANDS - 1]], axis=0))      # [58, N]
    x0trip = np.ascontiguousarray(np.tile(x0t, (3, 1)))       # [117, N]

    # compact Gram rhs source: x0c[(b',d'), t, j] = X0[t*8+b', j, d']
    x0c = np.ascontiguousarray(
        Xc.reshape(NT8, 8, F0, D).transpose(1, 3, 0, 2)       # [b', d, t, j]
        .reshape(128, NT8, F0).astype(BF16))

    # block-diagonal ones for S2: [(8b',16d), 8b] -> 1 iff b'==b
    onesbd = np.zeros((128, 8), dtype=BF16)
    for b in range(8):
        onesbd[b * D:(b + 1) * D, b] = 1

    return {
        "x0t": x0t, "x0dup": x0dup, "x0trip": x0trip,
        "w0": w0l, "w1": w1l, "w2": w2l, "x0c": x0c, "onesbd": onesbd,
        "b0": b0.reshape(128, 1).astype(np.float32),
        "b1": b1.reshape(128, 1).astype(np.float32),
    }


def _prep_weights(W0, W1, W2):
    # L0 symmetric bands: chunk c, band-in-chunk i -> delta = 3c+i,
    # row kk = i*39 + j pairs x0[j] (from x0trip) with x0[(j+delta)%39]
    # (from x0dup). Off-diagonal weights doubled (each unordered pair once).
    W0r = W0.reshape(H, F0, F0)
    W0sym = W0r + W0r.transpose(0, 2, 1)
    jj = np.arange(F0)
    w0l = np.zeros((128, C0_CHUNKS, 128), dtype=BF16)
    for delta in range(NBANDS):
        c, i = divmod(delta, 3)
        kk = i * F0 + jj
        kcol = (jj + delta) % F0
        vals = W0r[:, jj, jj] if delta == 0 else W0sym[:, jj, kcol]
        w0l[kk, c, :] = vals.T.astype(BF16)
    w1l = np.ascontiguousarray(
        W1.reshape(H, F0, 128).transpose(2, 1, 0).astype(BF16))
    w2l = np.ascontiguousarray(
        W2.reshape(H, F0, 128).transpose(2, 1, 0).astype(BF16))
    return w0l, w1l, w2l


def kernel(embedded_features, W0, b0, W1, b1, W2, b2):
    from concourse.bass_utils import run_bass_kernel_spmd

    X = np.asarray(embedded_features, dtype=np.float32)
    b0 = np.asarray(b0, dtype=np.float32)
    b1 = np.asarray(b1, dtype=np.float32)
    b2 = np.asarray(b2, dtype=np.float32)
    w0l, w1l, w2l = _prep_weights(np.asarray(W0, dtype=np.float32),
                                  np.asarray(W1, dtype=np.float32),
                                  np.asarray(W2, dtype=np.float32))

    if "nc" not in _CACHE:
        _CACHE["nc"] = _build()
    nc = _CACHE["nc"]

    in_maps = [
        _prep_core(X[c * BC:(c + 1) * BC], w0l, w1l, w2l, b0, b1)
        for c in range(N_CORES)
    ]
    res = run_bass_kernel_spmd(nc, in_maps, core_ids=list(range(N_CORES)))

    out = np.empty((B, 3 * H), dtype=np.float32)
    for c in range(N_CORES):
        r = res.results[c]
        sl = slice(c * BC, (c + 1) * BC)
        # s1/s2 already include the bias (added per-d on device: D*b total);
        # s3 is computed bias-free via the Gram trick, add D*b2 here.
        out[sl, 0:H] = r["s1"].T
        out[sl, H:2 * H] = r["s2"].T
        out[sl, 2 * H:3 * H] = r["s3"].T + D * b2[None, :]
    return out
